# revision 41
# baseline (speedup 1.0000x reference)
"""Trainium2 Bass kernel for causal multi-head attention with RoPE.

Sharding: tensor-parallel over heads. 16 heads / 8 cores = 2 heads per core.
Each core computes QKV projection for its 2 heads (full sequence), RoPE,
causal flash-style attention, and the output rows for its heads (the
reference's permute+reshape makes output rows partition cleanly by head).

v10: bf16 operands (1 cycle/row on PE, same as f32r, half DMA/SBUF), fp32
PSUM accumulation.  The attention blocks of token-block N-1 are WOVEN at
fine grain with the QKV chain matmuls of token-block N in emission order:
the in-order PE then always has Act-independent chain work queued between
each scores matmul and its exp-dependent attnV matmul, hiding the Act exp
latency (~650ns/block vs 426ns/block of PE attention work) and keeping the
PE p-state at 2.4GHz (any idle gap drops it to 1.2GHz for ~3us).
Causal masking happens on the PE itself (a -1e9 rank-full accumulating
matmul on the diagonal ramp) so attnV depends on nothing but the Act exp.
Softmax denominators: DVE running sum + one ones-matmul per query block.
out_w stays resident in SBUF (loaded once during the early steps).
"""

import math
import os
import sys

for _p in ("/opt/trn_rl_repo",):
    if _p not in sys.path and os.path.isdir(_p):
        sys.path.insert(0, _p)

import ml_dtypes
import numpy as np

import concourse.bass as bass  # noqa: F401  (AP helpers)
import concourse.mybir as mybir
import concourse.tile as tile
from concourse import bacc
from concourse.bass_utils import run_bass_kernel_spmd

F32 = mybir.dt.float32
BF16 = mybir.dt.bfloat16
NPBF = ml_dtypes.bfloat16

B, T, C = 2, 2048, 2048
H, D = 16, 128
N_CORES = 8
HPC = H // N_CORES          # heads per core (2)
BT = B * T                  # 4096
KC = C // 128               # 16 contraction blocks
TB = 512                    # token block (projection AND attention)
NTB = T // TB               # 4 t-blocks per batch
OJ = 512                    # out-proj column block
SCALE = 1.0 / math.sqrt(D)

_CACHED_NC = None


def build_nc():
    nc = bacc.Bacc("TRN2", target_bir_lowering=False)

    xT = nc.dram_tensor("xT", [C, BT], BF16, kind="ExternalInput")
    wqkT = nc.dram_tensor("wqkT", [C, 4 * 128], BF16, kind="ExternalInput")
    wvT = nc.dram_tensor("wvT", [C, 2 * 128], BF16, kind="ExternalInput")
    owF = nc.dram_tensor("owF", [C, C], BF16, kind="ExternalInput")
    cosF = nc.dram_tensor("cosF", [128, T], BF16, kind="ExternalInput")
    sinS = nc.dram_tensor("sinS", [128, T], BF16, kind="ExternalInput")
    onesI = nc.dram_tensor("onesI", [128, 128], BF16, kind="ExternalInput")
    maskT = nc.dram_tensor("maskT", [128, 128], BF16, kind="ExternalInput")
    negI = nc.dram_tensor("negI", [128, 128], BF16, kind="ExternalInput")
    y = nc.dram_tensor("y", [B * HPC, 128, C], BF16, kind="ExternalOutput")

    with tile.TileContext(nc) as tc:
        with tc.tile_pool(name="wpool", bufs=1) as wpool, \
             tc.tile_pool(name="xpool", bufs=6) as xpool, \
             tc.tile_pool(name="rotpool", bufs=2) as rotpool, \
             tc.tile_pool(name="vpool", bufs=2) as vpool, \
             tc.tile_pool(name="apool", bufs=2) as apool, \
             tc.tile_pool(name="epool", bufs=6) as epool, \
             tc.tile_pool(name="tpool", bufs=2) as tpool, \
             tc.tile_pool(name="dapool", bufs=2) as dapool, \
             tc.tile_pool(name="rpool", bufs=2) as rpool, \
             tc.tile_pool(name="ypool", bufs=2) as ypool, \
             tc.tile_pool(name="owpool", bufs=1) as owpool, \
             tc.tile_pool(name="flowps", bufs=3, space="PSUM") as flowps, \
             tc.tile_pool(name="attps", bufs=2, space="PSUM") as attps, \
             tc.tile_pool(name="qkps", bufs=2, space="PSUM") as qkps, \
             tc.tile_pool(name="vps", bufs=1, space="PSUM") as vps:

            twqk = wpool.tile([128, KC, 4 * 128], BF16)
            twv = wpool.tile([128, KC, 2 * 128], BF16)
            tcf = wpool.tile([128, T], BF16)
            tsn = wpool.tile([128, T], BF16)
            tones = wpool.tile([128, 128], BF16)
            tmsk = wpool.tile([128, 128], BF16)
            tnegI = wpool.tile([128, 128], BF16)
            wqkr = wqkT.rearrange("(kb p) m -> p kb m", p=128)
            wvr = wvT.rearrange("(kb p) m -> p kb m", p=128)
            owFr = owF[:, :].rearrange("(u p) j -> p u j", p=128)
            for k in range(4):
                nc.sync.dma_start(twqk[:, k, :], wqkr[:, k, :])

            # registries
            rots = {}     # (b, m, tb) -> [128, TB] bf16 tile
            vts = {}      # (b, tb) -> [128, 4, 256] bf16 tile
            atn = {}      # (b, h) -> [128, T] bf16 tile
            owtiles = {}  # jb -> resident owj tile
            pending = []  # deferred (den matmul, rcp, atn mul) closures
            prev_mm = []  # deferred attnV matmuls, 2-deep, cross-unit

            def flush_pending():
                while pending:
                    pending.pop(0)()

            def drain_att():
                while prev_mm:
                    prev_mm.pop(0)()
                flush_pending()

            def qkv_closures(b, tb):
                """DMA the x tiles now; return chain-chunk closures to be
                woven with the previous block's attention closures."""
                c0 = b * T + tb * TB
                ts_sl = slice(tb * TB, (tb + 1) * TB)
                first = (b == 0 and tb == 0)
                xTr = xT[:, c0:c0 + TB].rearrange("(kb p) t -> p kb t", p=128)
                xq = []
                for g in range(KC // 4):
                    xg = xpool.tile([128, 4, TB], BF16, tag="xk", name="xg")
                    nc.sync.dma_start(xg[:], xTr[:, g * 4:(g + 1) * 4, :])
                    xq.append(xg)
                    if first and g < 3:
                        # interleave remaining qk-weight blocks with x so the
                        # first chains are never starved
                        for k in range(4 * (g + 1), 4 * (g + 2)):
                            nc.sync.dma_start(twqk[:, k, :], wqkr[:, k, :])
                xk = [xq[k // 4][:, k % 4, :] for k in range(KC)]
                if first:
                    for k in range(KC):
                        nc.sync.dma_start(twv[:, k, :], wvr[:, k, :])
                    nc.sync.dma_start(tones[:], onesI[:, :])
                    nc.sync.dma_start(tmsk[:], maskT[:, :])
                    nc.sync.dma_start(tnegI[:], negI[:, :])
                if b == 0:
                    # just-in-time rope table slices
                    nc.sync.dma_start(tcf[:, ts_sl], cosF[:, ts_sl])
                    nc.sync.dma_start(tsn[:, ts_sl], sinS[:, ts_sl])

                state = {}

                def qk_chunk(m, cch):
                    if cch == 0:
                        state[m] = qkps.tile([128, TB], F32, tag="psqk",
                                             name="psqk")
                    ps = state[m]
                    for k in range(4 * cch, 4 * cch + 4):
                        nc.tensor.matmul(
                            ps[:], twqk[:, k, m * 128:(m + 1) * 128],
                            xk[k], start=(k == 0), stop=(k == KC - 1))

                def rope(m):
                    rt = rotpool.tile([128, TB], BF16, tag=f"rot{m}_{tb}",
                                      name=f"rot{m}_{tb}",
                                      bufs=1 if m < 2 else None)
                    rots[(b, m, tb)] = rt
                    ps = state[m]
                    # RoPE: rows 0:64 = x1, 64:128 = x2 of this head tensor
                    qsb = tpool.tile([128, TB], F32, tag="qsb", name="qsb")
                    nc.scalar.copy(qsb[:], ps[:])
                    qsw = tpool.tile([128, TB], F32, tag="qsw", name="qsw")
                    nc.gpsimd.dma_start(qsw[0:64, :], qsb[64:128, :])
                    nc.gpsimd.dma_start(qsw[64:128, :], qsb[0:64, :])
                    pc = tpool.tile([128, TB], F32, tag="pc", name="pc")
                    nc.vector.tensor_mul(out=pc[:], in0=qsb[:],
                                         in1=tcf[:, ts_sl])
                    pn = tpool.tile([128, TB], F32, tag="pn", name="pn")
                    nc.gpsimd.tensor_mul(out=pn[:], in0=qsw[:],
                                         in1=tsn[:, ts_sl])
                    nc.vector.tensor_add(out=rt[:], in0=pc[:], in1=pn[:])

                def v_chunk(ts, cch):
                    if ts == 0 and cch == 0:
                        vts[(b, tb)] = vpool.tile(
                            [128, 4, 2 * 128], BF16, tag=f"vt{tb}",
                            name=f"vt{tb}")
                        # both 128-token V chunks double-buffer inside one
                        # PSUM bank
                        state["vb"] = vps.tile([128, 2, 2 * 128], F32,
                                               tag="psv", name="vbank")
                    psv = state["vb"][:, ts % 2, :]
                    for k in range(4 * cch, 4 * cch + 4):
                        nc.tensor.matmul(
                            psv, xk[k][:, ts * 128:(ts + 1) * 128],
                            twv[:, k, :], start=(k == 0), stop=(k == KC - 1))

                def v_cast(ts):
                    nc.vector.tensor_copy(vts[(b, tb)][:, ts, :],
                                          state["vb"][:, ts % 2, :])

                out = []
                for i in range(4):
                    for cch in range(4):
                        out.append(lambda m=i, c=cch: qk_chunk(m, c))
                    out.append(lambda m=i: rope(m))
                    for cch in range(4):
                        out.append(lambda t=i, c=cch: v_chunk(t, c))
                    out.append(lambda t=i: v_cast(t))
                return out

            def att_closures(b, tb):
                """Per-head lists of attention block closures (deferred)."""
                heads = []
                for h in range(HPC):
                    blocks = []
                    ns = (tb + 1) * (TB // 128)
                    unit = {}

                    def block(si, h=h, ns=ns, unit=unit):
                        if si == 0:
                            if (b, h) not in atn:
                                atn[(b, h)] = apool.tile(
                                    [128, T], BF16, tag=f"attnT{h}",
                                    name=f"attnT{h}")
                            unit["ps_att"] = attps.tile(
                                [128, TB], F32, tag="psatt", name="psatt")
                            unit["da"] = dapool.tile(
                                [128, TB], BF16, tag="da", name="da")
                            unit["el"] = [None]
                        ps_att, da = unit["ps_att"], unit["da"]
                        ps_sc = flowps.tile([128, TB], F32, tag="flow",
                                            name="flow")
                        et = epool.tile([128, TB], BF16, tag="et", name="et")
                        diag = si >= ns - TB // 128
                        # cols < r of a diagonal block are fully masked: they
                        # are never computed nor read downstream
                        r = si * 128 - tb * TB if diag else 0
                        nc.tensor.matmul(
                            ps_sc[:, r:],
                            rots[(b, 2 + h, si // 4)][
                                :, (si % 4) * 128:(si % 4 + 1) * 128],
                            rots[(b, h, tb)][:, r:], start=True,
                            stop=not diag, skip_group_check=diag)
                        if diag:
                            # causal ramp masking ON THE PE: accumulate
                            # -1e9*mask into the 128-wide diagonal strip so
                            # exp produces exact zeros and attnV depends on
                            # nothing but the Act exp (DVE/Pool masking here
                            # stalls the PE behind adjacent rope queue work)
                            nc.tensor.matmul(
                                ps_sc[:, r:r + 128], tmsk[:], tnegI[:],
                                start=False, stop=True, skip_group_check=True)
                        nc.scalar.activation(
                            et[:, r:], ps_sc[:, r:],
                            mybir.ActivationFunctionType.Exp,
                            scale=SCALE)
                        # denominator running sum on DVE; the last block is
                        # folded in by a second accumulating ones-matmul,
                        # cutting the serial DVE tail
                        if si == 0:
                            nc.vector.tensor_copy(da[:], et[:])
                        elif si < ns - 1:
                            nc.vector.tensor_add(out=da[:, r:],
                                                 in0=da[:, r:],
                                                 in1=et[:, r:])
                        else:
                            unit["el"][0] = et
                        # 2-deep attnV pipeline ACROSS units: each attnV
                        # consumes the et from two blocks ago
                        if len(prev_mm) >= 2:
                            prev_mm.pop(0)()
                        if si == 2:
                            # previous unit's epilogue (den/rcp/mul)
                            flush_pending()

                        def attnv(ep=et, p=si, ps_att=ps_att, r=r,
                                  last=(si == ns - 1)):
                            nc.tensor.matmul(
                                ps_att[:, r:],
                                vts[(b, p // 4)][:, p % 4,
                                                 h * 128:(h + 1) * 128],
                                ep[:, r:], start=(p == 0), stop=last,
                                skip_group_check=(r > 0))
                        prev_mm.append(attnv)
                        if si == ns - 1:
                            ts_sl = slice(tb * TB, (tb + 1) * TB)

                            def epilogue(ps_att=ps_att, da=da, unit=unit,
                                         at=atn[(b, h)], ts_sl=ts_sl):
                                rl = TB - 128
                                ps_den = flowps.tile([128, TB], F32,
                                                     tag="flow", name="flow")
                                nc.tensor.matmul(ps_den[:], tones[:], da[:],
                                                 start=True, stop=False,
                                                 skip_group_check=True)
                                nc.tensor.matmul(ps_den[:, rl:], tones[:],
                                                 unit["el"][0][:, rl:],
                                                 start=False, stop=True,
                                                 skip_group_check=True)
                                rcp = rpool.tile([128, TB], F32, tag="rcp",
                                                 name="rcp")
                                nc.vector.reciprocal_approx_fast(
                                    out=rcp[:], in_=ps_den[:])
                                nc.vector.tensor_mul(out=at[:, ts_sl],
                                                     in0=ps_att[:],
                                                     in1=rcp[:])
                            pending.append(epilogue)

                    for si in range((tb + 1) * (TB // 128)):
                        blocks.append(lambda si=si, blk=block: blk(si))
                    heads.append(blocks)
                return heads

            def proj_head(b, h):
                """Out-proj chains for one head; returns closures."""
                out = []

                def chain(jb, h=h):
                    av = atn[(b, h)][:].rearrange("p (a u) -> p a u", u=16)
                    # psy lives in the qk-chain PSUM pool: no qk chains run
                    # while proj runs, and sharing the attention "flow" ring
                    # would serialize woven proj chains against scores tiles
                    psy = qkps.tile([128, OJ], F32, tag="psqk", name="psy")
                    for u in range(KC):
                        nc.tensor.matmul(
                            psy[:], av[:, :, u], owtiles[jb][:, u, :],
                            start=(u == 0), stop=(u == KC - 1))
                    ys = ypool.tile([128, OJ], BF16, tag="ys", name="ys")
                    nc.scalar.copy(ys[:], psy[:])
                    nc.sync.dma_start(
                        y[b * HPC + h, :, jb * OJ:(jb + 1) * OJ], ys[:])

                for jb in range(C // OJ):
                    out.append(lambda jb=jb: chain(jb))
                return out

            def weave(alist, clist):
                """Emit both closure lists interleaved, order-preserving,
                spreading clist evenly across alist."""
                if not alist:
                    for f in clist:
                        f()
                    return
                na, ncl = len(alist), len(clist)
                ci = 0
                for i, a in enumerate(alist):
                    a()
                    tgt = ((i + 1) * ncl) // na
                    while ci < tgt:
                        clist[ci]()
                        ci += 1
                while ci < ncl:
                    clist[ci]()
                    ci += 1

            # software pipeline: attention(s-1) woven at block granularity
            # with the qkv chains of step s, so the PE always has
            # Act-independent matmuls queued behind every exp-dependent one
            steps = [(s // NTB, s % NTB) for s in range(B * NTB)]
            for s in range(B * NTB + 1):
                alist, bq, tbq = [], None, None
                if s >= 1:
                    bq, tbq = steps[s - 1]
                    heads = att_closures(bq, tbq)
                    alist = heads[0] + heads[1]
                clist = qkv_closures(*steps[s]) if s < B * NTB else []
                if s == B * NTB:
                    # epilogue step: weave head-0's out-proj into head-1's
                    # attention (after si==2, where head-0's epilogue flushed)
                    h0, h1 = heads
                    for f in h0 + h1[:3]:
                        f()
                    weave(h1[3:], proj_head(bq, 0))
                    drain_att()
                    for f in proj_head(bq, 1):
                        f()
                else:
                    weave(alist, clist)
                    if tbq == NTB - 1:
                        drain_att()
                        for f in proj_head(bq, 0) + proj_head(bq, 1):
                            f()
                if s <= NTB - 1:
                    # resident out_w tiles, loaded during the early steps
                    # (after each step's own x DMAs, before the next step's)
                    jb = s
                    owj = owpool.tile([128, KC, OJ], BF16, tag=f"owj{jb}",
                                      name=f"owj{jb}")
                    nc.sync.dma_start(
                        owj[:], owFr[:, :, jb * OJ:(jb + 1) * OJ])
                    owtiles[jb] = owj
    nc.compile()
    return nc


def _get_nc():
    global _CACHED_NC
    if _CACHED_NC is None:
        _CACHED_NC = build_nc()
    return _CACHED_NC


def _rope_tables():
    pos = np.arange(T, dtype=np.float64)[:, None]
    div = np.exp(np.arange(0, D, 2, dtype=np.float64) *
                 (-math.log(10000.0) / D))
    ang = pos * div  # [T, 64]
    sinT = np.sin(ang).T.astype(np.float32)  # [64, T]
    cosT = np.cos(ang).T.astype(np.float32)
    cosF = np.ascontiguousarray(np.concatenate([cosT, cosT], axis=0))
    sinS = np.ascontiguousarray(np.concatenate([-sinT, sinT], axis=0))
    return cosF.astype(NPBF), sinS.astype(NPBF)


def make_in_maps(x, qkv_w, out_w):
    xT = np.ascontiguousarray(x.reshape(BT, C).T.astype(NPBF))
    owF = np.ascontiguousarray(out_w.T.astype(NPBF))
    cosF, sinS = _rope_tables()
    ones = np.ones((128, 128), dtype=NPBF)
    # maskT[j, ks] = 1 iff ks > j; (maskT^T @ -1e9*I)[ks, q] = -1e9*[ks > q]
    mskT = np.triu(np.ones((128, 128)), 1).astype(NPBF)
    negI = (np.eye(128) * -1e9).astype(NPBF)
    in_maps = []
    for c in range(N_CORES):
        h0, h1 = 2 * c, 2 * c + 1
        wqk = np.concatenate([
            qkv_w[h0 * D:(h0 + 1) * D],
            qkv_w[h1 * D:(h1 + 1) * D],
            qkv_w[C + h0 * D:C + (h0 + 1) * D],
            qkv_w[C + h1 * D:C + (h1 + 1) * D],
        ], axis=0)                       # [512, 2048]
        wv = np.concatenate([
            qkv_w[2 * C + h0 * D:2 * C + (h0 + 1) * D],
            qkv_w[2 * C + h1 * D:2 * C + (h1 + 1) * D],
        ], axis=0)                       # [256, 2048]
        in_maps.append({
            "xT": xT,
            "wqkT": np.ascontiguousarray(wqk.T.astype(NPBF)),
            "wvT": np.ascontiguousarray(wv.T.astype(NPBF)),
            "owF": owF,
            "cosF": cosF,
            "sinS": sinS,
            "onesI": ones,
            "maskT": mskT,
            "negI": negI,
        })
    return in_maps


def kernel(x, qkv_w, out_w, _trace=False, _trace_kwargs=None):
    x = np.asarray(x, dtype=np.float32)
    qkv_w = np.asarray(qkv_w, dtype=np.float32)
    out_w = np.asarray(out_w, dtype=np.float32)
    nc = _get_nc()
    in_maps = make_in_maps(x, qkv_w, out_w)
    kwargs = {}
    if _trace:
        kwargs["trace"] = True
        if _trace_kwargs:
            kwargs.update(_trace_kwargs)
    res = run_bass_kernel_spmd(nc, in_maps, core_ids=list(range(N_CORES)),
                               **kwargs)
    out = np.empty((B, T, C), dtype=np.float32)
    for c in range(N_CORES):
        yc = np.asarray(res.results[c]["y"]).astype(np.float32)
        for b in range(B):
            for hl in range(HPC):
                hg = HPC * c + hl
                out[b, hg * 128:(hg + 1) * 128] = yc[b * HPC + hl]
    if _trace:
        return out, res
    return out


# revision 43
# speedup vs baseline: 1.1394x; 1.1394x over previous
"""Trainium2 Bass kernel for causal multi-head attention with RoPE.

Sharding: tensor-parallel over heads. 16 heads / 8 cores = 2 heads per core.
Each core computes QKV projection for its 2 heads (full sequence), RoPE,
causal flash-style attention, and the output rows for its heads (the
reference's permute+reshape makes output rows partition cleanly by head).

v10: bf16 operands (1 cycle/row on PE, same as f32r, half DMA/SBUF), fp32
PSUM accumulation.  The attention blocks of token-block N-1 are WOVEN at
fine grain with the QKV chain matmuls of token-block N in emission order:
the in-order PE then always has Act-independent chain work queued between
each scores matmul and its exp-dependent attnV matmul, hiding the Act exp
latency (~650ns/block vs 426ns/block of PE attention work) and keeping the
PE p-state at 2.4GHz (any idle gap drops it to 1.2GHz for ~3us).
Causal masking happens on the PE itself (a -1e9 rank-full accumulating
matmul on the diagonal ramp) so attnV depends on nothing but the Act exp.
Softmax denominators: DVE running sum + one ones-matmul per query block.
out_w stays resident in SBUF (loaded once during the early steps).
"""

import math
import os
import sys

for _p in ("/opt/trn_rl_repo",):
    if _p not in sys.path and os.path.isdir(_p):
        sys.path.insert(0, _p)

import ml_dtypes
import numpy as np

import concourse.bass as bass  # noqa: F401  (AP helpers)
import concourse.mybir as mybir
import concourse.tile as tile
from concourse import bacc
from concourse.bass_utils import run_bass_kernel_spmd

F32 = mybir.dt.float32
BF16 = mybir.dt.bfloat16
NPBF = ml_dtypes.bfloat16

B, T, C = 2, 2048, 2048
H, D = 16, 128
N_CORES = 8
HPC = H // N_CORES          # heads per core (2)
BT = B * T                  # 4096
KC = C // 128               # 16 contraction blocks
TB = 512                    # token block (projection AND attention)
NTB = T // TB               # 4 t-blocks per batch
OJ = 512                    # out-proj column block
SCALE = 1.0 / math.sqrt(D)

_CACHED_NC = None


def build_nc():
    nc = bacc.Bacc("TRN2", target_bir_lowering=False)

    xT = nc.dram_tensor("xT", [C, BT], BF16, kind="ExternalInput")
    wqkT = nc.dram_tensor("wqkT", [C, 4 * 128], BF16, kind="ExternalInput")
    wvT = nc.dram_tensor("wvT", [C, 2 * 128], BF16, kind="ExternalInput")
    owF = nc.dram_tensor("owF", [C, C], BF16, kind="ExternalInput")
    cosF = nc.dram_tensor("cosF", [128, T], BF16, kind="ExternalInput")
    sinS = nc.dram_tensor("sinS", [128, T], BF16, kind="ExternalInput")
    onesI = nc.dram_tensor("onesI", [128, 128], BF16, kind="ExternalInput")
    maskT = nc.dram_tensor("maskT", [128, 128], BF16, kind="ExternalInput")
    negI = nc.dram_tensor("negI", [128, 128], BF16, kind="ExternalInput")
    y = nc.dram_tensor("y", [B * HPC, 128, C], BF16, kind="ExternalOutput")

    with tile.TileContext(nc) as tc:
        with tc.tile_pool(name="wpool", bufs=1) as wpool, \
             tc.tile_pool(name="xpool", bufs=6) as xpool, \
             tc.tile_pool(name="rotpool", bufs=2) as rotpool, \
             tc.tile_pool(name="vpool", bufs=2) as vpool, \
             tc.tile_pool(name="apool", bufs=2) as apool, \
             tc.tile_pool(name="epool", bufs=6) as epool, \
             tc.tile_pool(name="tpool", bufs=2) as tpool, \
             tc.tile_pool(name="dapool", bufs=2) as dapool, \
             tc.tile_pool(name="rpool", bufs=2) as rpool, \
             tc.tile_pool(name="ypool", bufs=2) as ypool, \
             tc.tile_pool(name="owpool", bufs=1) as owpool, \
             tc.tile_pool(name="flowps", bufs=3, space="PSUM") as flowps, \
             tc.tile_pool(name="attps", bufs=2, space="PSUM") as attps, \
             tc.tile_pool(name="qkps", bufs=2, space="PSUM") as qkps, \
             tc.tile_pool(name="vps", bufs=1, space="PSUM") as vps:

            twqk = wpool.tile([128, KC, 4 * 128], BF16)
            twv = wpool.tile([128, KC, 2 * 128], BF16)
            tcf = wpool.tile([128, T], BF16)
            tsn = wpool.tile([128, T], BF16)
            tones = wpool.tile([128, 128], BF16)
            tmsk = wpool.tile([128, 128], BF16)
            tnegI = wpool.tile([128, 128], BF16)
            wqkr = wqkT.rearrange("(kb p) m -> p kb m", p=128)
            wvr = wvT.rearrange("(kb p) m -> p kb m", p=128)
            owFr = owF[:, :].rearrange("(u p) j -> p u j", p=128)
            for k in range(4):
                nc.sync.dma_start(twqk[:, k, :], wqkr[:, k, :])

            # registries
            rots = {}     # (b, m, tb) -> [128, TB] bf16 tile
            vts = {}      # (b, tb) -> [128, 4, 256] bf16 tile
            atn = {}      # (b, h) -> [128, T] bf16 tile
            owtiles = {}  # jb -> resident owj tile
            pending = []  # deferred (den matmul, rcp, atn mul) closures
            prev_mm = []  # deferred attnV matmuls, 2-deep, cross-unit

            def flush_pending():
                while pending:
                    pending.pop(0)()

            def drain_att():
                while prev_mm:
                    prev_mm.pop(0)()
                flush_pending()

            def qkv_closures(b, tb):
                """DMA the x tiles now; return chain-chunk closures to be
                woven with the previous block's attention closures."""
                c0 = b * T + tb * TB
                ts_sl = slice(tb * TB, (tb + 1) * TB)
                first = (b == 0 and tb == 0)
                xTr = xT[:, c0:c0 + TB].rearrange("(kb p) t -> p kb t", p=128)
                xq = []
                for g in range(KC // 4):
                    xg = xpool.tile([128, 4, TB], BF16, tag="xk", name="xg")
                    nc.sync.dma_start(xg[:], xTr[:, g * 4:(g + 1) * 4, :])
                    xq.append(xg)
                    if first and g < 3:
                        # interleave remaining qk-weight blocks with x so the
                        # first chains are never starved
                        for k in range(4 * (g + 1), 4 * (g + 2)):
                            nc.sync.dma_start(twqk[:, k, :], wqkr[:, k, :])
                xk = [xq[k // 4][:, k % 4, :] for k in range(KC)]
                if first:
                    for k in range(KC):
                        nc.sync.dma_start(twv[:, k, :], wvr[:, k, :])
                    nc.sync.dma_start(tones[:], onesI[:, :])
                    nc.sync.dma_start(tmsk[:], maskT[:, :])
                    nc.sync.dma_start(tnegI[:], negI[:, :])
                if b == 0:
                    # just-in-time rope table slices
                    nc.sync.dma_start(tcf[:, ts_sl], cosF[:, ts_sl])
                    nc.sync.dma_start(tsn[:, ts_sl], sinS[:, ts_sl])

                state = {}

                def qk_chunk(m, cch):
                    if cch == 0:
                        state[m] = qkps.tile([128, TB], F32, tag="psqk",
                                             name="psqk")
                    ps = state[m]
                    for k in range(4 * cch, 4 * cch + 4):
                        nc.tensor.matmul(
                            ps[:], twqk[:, k, m * 128:(m + 1) * 128],
                            xk[k], start=(k == 0), stop=(k == KC - 1))

                def rope(m):
                    rt = rotpool.tile([128, TB], BF16, tag=f"rot{m}_{tb}",
                                      name=f"rot{m}_{tb}",
                                      bufs=1 if m < 2 else None)
                    rots[(b, m, tb)] = rt
                    ps = state[m]
                    # RoPE: rows 0:64 = x1, 64:128 = x2 of this head tensor
                    qsb = tpool.tile([128, TB], F32, tag="qsb", name="qsb")
                    nc.scalar.copy(qsb[:], ps[:])
                    qsw = tpool.tile([128, TB], F32, tag="qsw", name="qsw")
                    nc.gpsimd.dma_start(qsw[0:64, :], qsb[64:128, :])
                    nc.gpsimd.dma_start(qsw[64:128, :], qsb[0:64, :])
                    pc = tpool.tile([128, TB], F32, tag="pc", name="pc")
                    nc.vector.tensor_mul(out=pc[:], in0=qsb[:],
                                         in1=tcf[:, ts_sl])
                    pn = tpool.tile([128, TB], F32, tag="pn", name="pn")
                    nc.gpsimd.tensor_mul(out=pn[:], in0=qsw[:],
                                         in1=tsn[:, ts_sl])
                    nc.vector.tensor_add(out=rt[:], in0=pc[:], in1=pn[:])

                def v_chunk(ts, cch):
                    if ts == 0 and cch == 0:
                        vts[(b, tb)] = vpool.tile(
                            [128, 4, 2 * 128], BF16, tag=f"vt{tb}",
                            name=f"vt{tb}")
                        # both 128-token V chunks double-buffer inside one
                        # PSUM bank
                        state["vb"] = vps.tile([128, 2, 2 * 128], F32,
                                               tag="psv", name="vbank")
                    psv = state["vb"][:, ts % 2, :]
                    for k in range(4 * cch, 4 * cch + 4):
                        nc.tensor.matmul(
                            psv, xk[k][:, ts * 128:(ts + 1) * 128],
                            twv[:, k, :], start=(k == 0), stop=(k == KC - 1))

                def v_cast(ts):
                    # on Act: the DVE queue is full of woven attention work
                    # (da adds), which would delay freeing the V PSUM bank
                    nc.scalar.copy(vts[(b, tb)][:, ts, :],
                                   state["vb"][:, ts % 2, :])

                out = []
                for i in range(4):
                    for cch in range(4):
                        out.append(lambda m=i, c=cch: qk_chunk(m, c))
                    out.append(lambda m=i: rope(m))
                    for cch in range(4):
                        out.append(lambda t=i, c=cch: v_chunk(t, c))
                    out.append(lambda t=i: v_cast(t))
                return out

            def att_closures(b, tb):
                """Per-head lists of attention block closures (deferred)."""
                heads = []
                for h in range(HPC):
                    blocks = []
                    ns = (tb + 1) * (TB // 128)
                    unit = {}

                    def block(si, h=h, ns=ns, unit=unit):
                        if si == 0:
                            if (b, h) not in atn:
                                atn[(b, h)] = apool.tile(
                                    [128, T], BF16, tag=f"attnT{h}",
                                    name=f"attnT{h}")
                            unit["ps_att"] = attps.tile(
                                [128, TB], F32, tag="psatt", name="psatt")
                            unit["da"] = dapool.tile(
                                [128, TB], BF16, tag="da", name="da")
                            unit["el"] = [None]
                        ps_att, da = unit["ps_att"], unit["da"]
                        ps_sc = flowps.tile([128, TB], F32, tag="flow",
                                            name="flow")
                        et = epool.tile([128, TB], BF16, tag="et", name="et")
                        diag = si >= ns - TB // 128
                        # cols < r of a diagonal block are fully masked: they
                        # are never computed nor read downstream
                        r = si * 128 - tb * TB if diag else 0
                        nc.tensor.matmul(
                            ps_sc[:, r:],
                            rots[(b, 2 + h, si // 4)][
                                :, (si % 4) * 128:(si % 4 + 1) * 128],
                            rots[(b, h, tb)][:, r:], start=True,
                            stop=not diag, skip_group_check=diag)
                        if diag:
                            # causal ramp masking ON THE PE: accumulate
                            # -1e9*mask into the 128-wide diagonal strip so
                            # exp produces exact zeros and attnV depends on
                            # nothing but the Act exp (DVE/Pool masking here
                            # stalls the PE behind adjacent rope queue work)
                            nc.tensor.matmul(
                                ps_sc[:, r:r + 128], tmsk[:], tnegI[:],
                                start=False, stop=True, skip_group_check=True)
                        nc.scalar.activation(
                            et[:, r:], ps_sc[:, r:],
                            mybir.ActivationFunctionType.Exp,
                            scale=SCALE)
                        # denominator running sum on DVE; the last block is
                        # folded in by a second accumulating ones-matmul,
                        # cutting the serial DVE tail
                        if si == 0:
                            nc.vector.tensor_copy(da[:], et[:])
                        elif si < ns - 1:
                            nc.vector.tensor_add(out=da[:, r:],
                                                 in0=da[:, r:],
                                                 in1=et[:, r:])
                        else:
                            unit["el"][0] = et
                        # 2-deep attnV pipeline ACROSS units: each attnV
                        # consumes the et from two blocks ago
                        if len(prev_mm) >= 2:
                            prev_mm.pop(0)()
                        if si == 2:
                            # previous unit's epilogue (den/rcp/mul)
                            flush_pending()

                        def attnv(ep=et, p=si, ps_att=ps_att, r=r,
                                  last=(si == ns - 1)):
                            nc.tensor.matmul(
                                ps_att[:, r:],
                                vts[(b, p // 4)][:, p % 4,
                                                 h * 128:(h + 1) * 128],
                                ep[:, r:], start=(p == 0), stop=last,
                                skip_group_check=(r > 0))
                        prev_mm.append(attnv)
                        if si == ns - 1:
                            ts_sl = slice(tb * TB, (tb + 1) * TB)

                            def epilogue(ps_att=ps_att, da=da, unit=unit,
                                         at=atn[(b, h)], ts_sl=ts_sl):
                                rl = TB - 128
                                ps_den = flowps.tile([128, TB], F32,
                                                     tag="flow", name="flow")
                                nc.tensor.matmul(ps_den[:], tones[:], da[:],
                                                 start=True, stop=False,
                                                 skip_group_check=True)
                                nc.tensor.matmul(ps_den[:, rl:], tones[:],
                                                 unit["el"][0][:, rl:],
                                                 start=False, stop=True,
                                                 skip_group_check=True)
                                rcp = rpool.tile([128, TB], F32, tag="rcp",
                                                 name="rcp")
                                nc.vector.reciprocal_approx_fast(
                                    out=rcp[:], in_=ps_den[:])
                                nc.vector.tensor_mul(out=at[:, ts_sl],
                                                     in0=ps_att[:],
                                                     in1=rcp[:])
                            pending.append(epilogue)

                    for si in range((tb + 1) * (TB // 128)):
                        blocks.append(lambda si=si, blk=block: blk(si))
                    heads.append(blocks)
                return heads

            def proj_head(b, h):
                """Out-proj chains for one head; returns closures."""
                out = []

                def chain(jb, h=h):
                    av = atn[(b, h)][:].rearrange("p (a u) -> p a u", u=16)
                    # psy lives in the qk-chain PSUM pool: no qk chains run
                    # while proj runs, and sharing the attention "flow" ring
                    # would serialize woven proj chains against scores tiles
                    psy = qkps.tile([128, OJ], F32, tag="psqk", name="psy")
                    for u in range(KC):
                        nc.tensor.matmul(
                            psy[:], av[:, :, u], owtiles[jb][:, u, :],
                            start=(u == 0), stop=(u == KC - 1))
                    ys = ypool.tile([128, OJ], BF16, tag="ys", name="ys")
                    nc.scalar.copy(ys[:], psy[:])
                    nc.sync.dma_start(
                        y[b * HPC + h, :, jb * OJ:(jb + 1) * OJ], ys[:])

                for jb in range(C // OJ):
                    out.append(lambda jb=jb: chain(jb))
                return out

            def weave(alist, clist):
                """Emit both closure lists interleaved, order-preserving,
                spreading clist evenly across alist."""
                if not alist:
                    for f in clist:
                        f()
                    return
                na, ncl = len(alist), len(clist)
                ci = 0
                for i, a in enumerate(alist):
                    a()
                    tgt = ((i + 1) * ncl) // na
                    while ci < tgt:
                        clist[ci]()
                        ci += 1
                while ci < ncl:
                    clist[ci]()
                    ci += 1

            # software pipeline: attention(s-1) woven at block granularity
            # with the qkv chains of step s, so the PE always has
            # Act-independent matmuls queued behind every exp-dependent one
            steps = [(s // NTB, s % NTB) for s in range(B * NTB)]
            for s in range(B * NTB + 1):
                alist, bq, tbq = [], None, None
                if s >= 1:
                    bq, tbq = steps[s - 1]
                    heads = att_closures(bq, tbq)
                    alist = heads[0] + heads[1]
                clist = qkv_closures(*steps[s]) if s < B * NTB else []
                if s == B * NTB:
                    # epilogue step: weave head-0's out-proj into head-1's
                    # attention (after si==2, where head-0's epilogue flushed)
                    h0, h1 = heads
                    for f in h0 + h1[:3]:
                        f()
                    weave(h1[3:], proj_head(bq, 0))
                    drain_att()
                    for f in proj_head(bq, 1):
                        f()
                else:
                    weave(alist, clist)
                    if tbq == NTB - 1:
                        drain_att()
                        for f in proj_head(bq, 0) + proj_head(bq, 1):
                            f()
                if s <= NTB - 1:
                    # resident out_w tiles, loaded during the early steps
                    # (after each step's own x DMAs, before the next step's)
                    jb = s
                    owj = owpool.tile([128, KC, OJ], BF16, tag=f"owj{jb}",
                                      name=f"owj{jb}")
                    nc.sync.dma_start(
                        owj[:], owFr[:, :, jb * OJ:(jb + 1) * OJ])
                    owtiles[jb] = owj
    nc.compile()
    return nc


def _get_nc():
    global _CACHED_NC
    if _CACHED_NC is None:
        _CACHED_NC = build_nc()
    return _CACHED_NC


def _rope_tables():
    pos = np.arange(T, dtype=np.float64)[:, None]
    div = np.exp(np.arange(0, D, 2, dtype=np.float64) *
                 (-math.log(10000.0) / D))
    ang = pos * div  # [T, 64]
    sinT = np.sin(ang).T.astype(np.float32)  # [64, T]
    cosT = np.cos(ang).T.astype(np.float32)
    cosF = np.ascontiguousarray(np.concatenate([cosT, cosT], axis=0))
    sinS = np.ascontiguousarray(np.concatenate([-sinT, sinT], axis=0))
    return cosF.astype(NPBF), sinS.astype(NPBF)


def make_in_maps(x, qkv_w, out_w):
    xT = np.ascontiguousarray(x.reshape(BT, C).T.astype(NPBF))
    owF = np.ascontiguousarray(out_w.T.astype(NPBF))
    cosF, sinS = _rope_tables()
    ones = np.ones((128, 128), dtype=NPBF)
    # maskT[j, ks] = 1 iff ks > j; (maskT^T @ -1e9*I)[ks, q] = -1e9*[ks > q]
    mskT = np.triu(np.ones((128, 128)), 1).astype(NPBF)
    negI = (np.eye(128) * -1e9).astype(NPBF)
    in_maps = []
    for c in range(N_CORES):
        h0, h1 = 2 * c, 2 * c + 1
        wqk = np.concatenate([
            qkv_w[h0 * D:(h0 + 1) * D],
            qkv_w[h1 * D:(h1 + 1) * D],
            qkv_w[C + h0 * D:C + (h0 + 1) * D],
            qkv_w[C + h1 * D:C + (h1 + 1) * D],
        ], axis=0)                       # [512, 2048]
        wv = np.concatenate([
            qkv_w[2 * C + h0 * D:2 * C + (h0 + 1) * D],
            qkv_w[2 * C + h1 * D:2 * C + (h1 + 1) * D],
        ], axis=0)                       # [256, 2048]
        in_maps.append({
            "xT": xT,
            "wqkT": np.ascontiguousarray(wqk.T.astype(NPBF)),
            "wvT": np.ascontiguousarray(wv.T.astype(NPBF)),
            "owF": owF,
            "cosF": cosF,
            "sinS": sinS,
            "onesI": ones,
            "maskT": mskT,
            "negI": negI,
        })
    return in_maps


def kernel(x, qkv_w, out_w, _trace=False, _trace_kwargs=None):
    x = np.asarray(x, dtype=np.float32)
    qkv_w = np.asarray(qkv_w, dtype=np.float32)
    out_w = np.asarray(out_w, dtype=np.float32)
    nc = _get_nc()
    in_maps = make_in_maps(x, qkv_w, out_w)
    kwargs = {}
    if _trace:
        kwargs["trace"] = True
        if _trace_kwargs:
            kwargs.update(_trace_kwargs)
    res = run_bass_kernel_spmd(nc, in_maps, core_ids=list(range(N_CORES)),
                               **kwargs)
    out = np.empty((B, T, C), dtype=np.float32)
    for c in range(N_CORES):
        yc = np.asarray(res.results[c]["y"]).astype(np.float32)
        for b in range(B):
            for hl in range(HPC):
                hg = HPC * c + hl
                out[b, hg * 128:(hg + 1) * 128] = yc[b * HPC + hl]
    if _trace:
        return out, res
    return out
